# revision 34
# baseline (speedup 1.0000x reference)
"""Masked cross-attention (B=4, NQ=100, HW=4096, D=1024, H=16) on 8 TRN2 cores.

Sharding: kv rows (keys) are split 8 ways; each core runs LayerNorm + K/V
projection on its 512-key slice per batch, computes unnormalized partial
attention for all (b, h) against its keys, all-reduces the softmax
denominators on device, normalizes, and computes a partial out-projection.
The host sums the 8 partial outputs and adds the folded bias.

v2 schedule: the kernel is split so the denominator all-reduce overlaps
compute. Phase 1 (per batch): kv LN + transpose, K projection, scores,
exp, mask, and the denominator row-sums (matmul against a ones column).
After batch 3's denominators are DMA'd the all-reduce is kicked. Phase 2
(V projection + context matmuls, ~80us of tensor work) runs while the
collective is in flight. The tail normalizes the context by the global
reciprocals (broadcast across partitions via a selector matmul) and runs
the out-projection.
"""
import sys

sys.path.insert(0, "/opt/trn_rl_repo")

import numpy as np
import ml_dtypes

import concourse.bacc as bacc
import concourse.bass as bass
import concourse.mybir as mybir
import concourse.tile as tile
from concourse.bass_utils import run_bass_kernel_spmd
from concourse.masks import make_identity

B, NQ, HW, D, H = 4, 100, 4096, 1024, 16
HD = D // H          # 64
NCORE = 8
KC = HW // NCORE     # 512 keys per core per batch
NKT = KC // 128      # 4 key sub-tiles of 128
NDC = D // 128       # 8 chunks of the model dim
EPS = 1e-5
SCALE = 1.0 / np.sqrt(np.float32(HD))  # 1/8

F32 = mybir.dt.float32
BF16 = mybir.dt.bfloat16
AF = mybir.ActivationFunctionType
ALU = mybir.AluOpType

_compiled = {}


def _build():
    nc = bacc.Bacc("TRN2", target_bir_lowering=False, num_devices=NCORE)

    kv_d = nc.dram_tensor("kv", [B, NKT, 128, D], BF16, kind="ExternalInput")
    q_d = nc.dram_tensor("q", [B, NQ, D], BF16, kind="ExternalInput")
    mask_d = nc.dram_tensor("maskT", [B, 128, NKT, NQ], BF16, kind="ExternalInput")
    wq_d = nc.dram_tensor("wqT", [128, NDC, D], BF16, kind="ExternalInput")
    wk_d = nc.dram_tensor("wkT", [128, NDC, D], BF16, kind="ExternalInput")
    wv_d = nc.dram_tensor("wvT", [128, NDC, D], BF16, kind="ExternalInput")
    wo_d = nc.dram_tensor("woT", [128, NDC, D], BF16, kind="ExternalInput")
    bq_d = nc.dram_tensor("biasq", [128, NDC], F32, kind="ExternalInput")
    bk_d = nc.dram_tensor("biask", [128, NDC], F32, kind="ExternalInput")
    sel_d = nc.dram_tensor("sel", [128, 2, NDC, 128], BF16, kind="ExternalInput")
    ident_d = nc.dram_tensor("ident", [128, 128], BF16, kind="ExternalInput")
    out_d = nc.dram_tensor("out", [128, NDC, B, NQ], BF16, kind="ExternalOutput")

    with tile.TileContext(nc) as tc:
        with (
            tc.tile_pool(name="sb", bufs=1) as sb,
            tc.tile_pool(name="ps", bufs=1, space="PSUM") as ps,
            tc.tile_pool(name="dram", bufs=1, space="DRAM") as dram,
        ):
            # ---- constants ----
            ident = sb.tile([128, 128], BF16, tag="ident")
            nc.sync.dma_start(ident[:], ident_d[:])
            eps_t = sb.tile([128, 1], F32, tag="eps")
            nc.vector.memset(eps_t[:], EPS)
            ones1 = sb.tile([128, 1], BF16, tag="ones1")
            nc.vector.memset(ones1[:], 1.0)
            # reciprocal-broadcast rhs: partition 32*(b%2)+h, free (b//2, q);
            # unused partition rows stay 1.0 (sel zeroes their contribution;
            # avoids 0*NaN in the broadcast matmul)
            rec_bf = sb.tile([128, 2, NQ], BF16, tag="rec_in", name="rec_bf")
            nc.vector.memset(rec_bf[:], 1.0)

            # ---- weights ----
            wk_sb = sb.tile([128, NDC, D], BF16, tag="wk")
            wv_sb = sb.tile([128, NDC, D], BF16, tag="wv")
            wq_sb = sb.tile([128, NDC, D], BF16, tag="wq", bufs=1, name="wq")
            wo_sb = sb.tile([128, NDC, D], BF16, tag="wo")
            bqv_sb = sb.tile([128, NDC], F32, tag="bqv")
            bkv_sb = sb.tile([128, NDC], F32, tag="bkv")
            bq_sb = [bqv_sb[:, j:j + 1] for j in range(NDC)]
            bk_sb = [bkv_sb[:, j:j + 1] for j in range(NDC)]
            sel_sb = sb.tile([128, 2, NDC, 128], BF16, tag="sel")

            # persistent activations
            kvnT = [
                sb.tile([128, NDC, NKT, 128], BF16, tag=f"kvnT{b}",
                        name=f"kvnT_{b}")
                for b in range(B)
            ]
            exp_all = [
                sb.tile([128, NKT, H, NQ], BF16, tag=f"exp{b}", name=f"exp_{b}")
                for b in range(B)
            ]
            qnT = sb.tile([128, NDC, B, NQ], BF16, tag="qnT", name="qnT")
            qpT = sb.tile([128, NDC, B, 2, NQ], BF16, tag="qpT", name="qpT")
            kpT = sb.tile([128, NDC, KC], BF16, tag="kpT", name="kpT0")

            den_all = sb.tile([NQ, B, H], F32, tag="den_all")
            sloc = dram.tile([B, NQ, H], F32)
            sglob = dram.tile([B, NQ, H], F32, tag="sglob")

            def layernorm_to_bf16(x, xn_bf16, p):
                """(x - mean) * rsqrt(var + eps), row-wise over the free dim."""
                stats = sb.tile([128, 2, 6], F32, tag="lnstats", bufs=4)
                nc.vector.bn_stats(stats[:p, 0, :], x[:p, 0:512])
                nc.vector.bn_stats(stats[:p, 1, :], x[:p, 512:1024])
                mv = sb.tile([128, 2], F32, tag="lnmv", bufs=4)
                nc.vector.bn_aggr(mv[:p], stats[:p])
                rstd = sb.tile([128, 1], F32, tag="lnrstd", bufs=4)
                nc.scalar.activation(rstd[:p], mv[:p, 1:2], AF.Sqrt, bias=eps_t[:p])
                nc.vector.reciprocal(rstd[:p], rstd[:p])
                nc.vector.tensor_scalar(
                    xn_bf16[:p], x[:p], mv[:p, 0:1], rstd[:p],
                    ALU.subtract, ALU.mult,
                )

            def prep_ln(b):
                """kv load + LN + transpose for batch b."""
                for r in range(NKT):
                    kvraw = sb.tile([128, D], BF16, tag="kvraw", bufs=4)
                    nc.sync.dma_start(kvraw[:], kv_d[b, r])
                    xn = sb.tile([128, D], BF16, tag="xn", bufs=2)
                    layernorm_to_bf16(kvraw, xn, 128)
                    for k4 in range(NDC // 4):
                        tr = ps.tile([128, 4, 128], BF16, tag="tr", bufs=2)
                        for kk in range(4):
                            k = 4 * k4 + kk
                            nc.tensor.transpose(
                                tr[:, kk, :], xn[:, k * 128:(k + 1) * 128], ident[:]
                            )
                        nc.scalar.activation(
                            kvnT[b][:, 4 * k4:4 * k4 + 4, r, :], tr[:], AF.Copy,
                        )

            def kproj(b):
                # K projection -> kpT[:, j, :]: [128 dout, KC keys] (+bias)
                for j in range(NDC):
                    acc = ps.tile([128, KC], F32, tag="mm", bufs=2)
                    for k in range(NDC):
                        nc.tensor.matmul(
                            acc[:],
                            lhsT=wk_sb[:, k, j * 128:(j + 1) * 128],
                            rhs=kvnT[b][:, k, :, :].rearrange("p r c -> p (r c)"),
                            start=(k == 0), stop=(k == NDC - 1),
                        )
                    nc.scalar.activation(
                        kpT[:, j, :], acc[:], AF.Identity, bias=bk_sb[j][:]
                    )

            def q_pipeline():
                """LayerNorm + transpose + projection of q, all batches."""
                for b in range(B):
                    qraw = sb.tile([NQ, D], BF16, tag="kvraw", bufs=4)
                    nc.sync.dma_start(qraw[:], q_d[b])
                    qn = sb.tile([NQ, D], BF16, tag="xn", bufs=2)
                    layernorm_to_bf16(qraw, qn, NQ)
                    for k4 in range(NDC // 4):
                        tr = ps.tile([128, 4, NQ], BF16, tag="tr", bufs=2)
                        for kk in range(4):
                            k = 4 * k4 + kk
                            nc.tensor.transpose(
                                tr[:, kk, :], qn[:, k * 128:(k + 1) * 128],
                                ident[:NQ, :NQ],
                            )
                        nc.vector.tensor_copy(
                            out=qnT[:, 4 * k4:4 * k4 + 4, b, :], in_=tr[:]
                        )
                # qpT[:, j]: [128, B, 2, NQ] block-diagonal by head: rows 0:64
                # hold head 2j over i=0 columns, rows 64:128 hold head 2j+1
                # over i=1 columns, zeros elsewhere.
                nc.vector.memset(qpT[:], 0.0)
                for j in range(NDC):
                    acc = ps.tile([128, 512], F32, tag="mm", bufs=2)
                    for k in range(NDC):
                        nc.tensor.matmul(
                            acc[:, 0:B * NQ],
                            lhsT=wq_sb[:, k, j * 128:(j + 1) * 128],
                            rhs=qnT[:, k, :, :].rearrange("p b q -> p (b q)"),
                            start=(k == 0), stop=(k == NDC - 1),
                        )
                    nc.scalar.activation(
                        qpT[0:HD, j, :, 0, :],
                        acc[0:HD, 0:B * NQ].rearrange("p (b q) -> p b q", b=B),
                        AF.Identity, bias=bq_sb[j][0:HD],
                    )
                    nc.scalar.activation(
                        qpT[HD:128, j, :, 1, :],
                        acc[HD:128, 0:B * NQ].rearrange("p (b q) -> p b q", b=B),
                        AF.Identity, bias=bq_sb[j][HD:128],
                    )

            def scores_block(b, mask_b):
                """scores^T, exp, mask, denominator row-sums for batch b."""
                for j in range(NDC):
                    for c2 in range(2):
                        sc = ps.tile([128, 2, 2, NQ], F32, tag="sc", bufs=2)
                        for cc in range(2):
                            c = 2 * c2 + cc
                            nc.tensor.matmul(
                                sc[:, cc, :, :].rearrange("p i q -> p (i q)"),
                                lhsT=kpT[:, j, c * 128:(c + 1) * 128],
                                rhs=qpT[:, j, b, :, :].rearrange(
                                    "p i q -> p (i q)"),
                                start=True, stop=True,
                            )
                        nc.scalar.activation(
                            exp_all[b][:, 2 * c2:2 * c2 + 2, 2 * j:2 * j + 2, :],
                            sc[:], AF.Exp, scale=float(SCALE),
                        )
                    for hh in range(2):
                        h = 2 * j + hh
                        nc.vector.tensor_mul(
                            exp_all[b][:, :, h, :], exp_all[b][:, :, h, :],
                            mask_b[:],
                        )
                    if j % 2 == 1:
                        # heads 4g..4g+3 masked: pre-sum the 4 key-subtiles
                        # on DVE, then one [NQ,1] matmul per head
                        g = j // 2
                        s01 = sb.tile([128, 4, NQ], BF16, tag="csum", bufs=4)
                        nc.gpsimd.tensor_tensor(
                            out=s01[:], in0=exp_all[b][:, 0, 4 * g:4 * g + 4, :],
                            in1=exp_all[b][:, 1, 4 * g:4 * g + 4, :],
                            op=ALU.add,
                        )
                        s23 = sb.tile([128, 4, NQ], BF16, tag="csum", bufs=4)
                        nc.gpsimd.tensor_tensor(
                            out=s23[:], in0=exp_all[b][:, 2, 4 * g:4 * g + 4, :],
                            in1=exp_all[b][:, 3, 4 * g:4 * g + 4, :],
                            op=ALU.add,
                        )
                        s = sb.tile([128, 4, NQ], BF16, tag="csum", bufs=4)
                        nc.gpsimd.tensor_tensor(
                            out=s[:], in0=s01[:], in1=s23[:], op=ALU.add,
                        )
                        for i in range(4):
                            den_q = ps.tile([NQ, 1], F32, tag="den", bufs=2)
                            nc.tensor.matmul(
                                den_q[:], lhsT=s[:, i, :], rhs=ones1[:],
                                start=True, stop=True,
                            )
                            nc.scalar.activation(
                                den_all[:, b, 4 * g + i:4 * g + i + 1],
                                den_q[:], AF.Copy,
                            )
                nc.sync.dma_start(sloc[b], den_all[:, b, :])

            # ---- phase 1: everything the denominators need ----
            mask0 = sb.tile([128, NKT, NQ], BF16, tag="maskb", bufs=2)
            nc.sync.dma_start(mask0[:], mask_d[0])
            masks = {0: mask0}

            def loads_after(b):
                # staggered weight loads on the sync queue, after each
                # batch's kv so the critical kv tiles lead
                if b == 0:
                    nc.sync.dma_start(wk_sb[:], wk_d[:])
                    nc.sync.dma_start(bkv_sb[:], bk_d[:])
                    nc.sync.dma_start(wq_sb[:], wq_d[:])
                    nc.sync.dma_start(bqv_sb[:], bq_d[:])
                elif b == 1:
                    nc.sync.dma_start(wv_sb[:], wv_d[:])
                elif b == 2:
                    nc.sync.dma_start(wo_sb[:], wo_d[:])
                    nc.sync.dma_start(sel_sb[:], sel_d[:])

            def load_mask(b):
                m = sb.tile([128, NKT, NQ], BF16, tag="maskb", bufs=2)
                nc.sync.dma_start(m[:], mask_d[b])
                masks[b] = m

            prep_ln(0)
            prep_ln(1)          # kv1 rides the ring before the weights so
            loads_after(0)      # its transposes cover the wk wait
            kproj(0)
            load_mask(1)
            q_pipeline()
            scores_block(0, masks.pop(0))
            kproj(1)
            load_mask(2)
            scores_block(1, masks.pop(1))
            # first half all-reduce (batches 0,1) overlaps phase 1
            nc.gpsimd.collective_compute(
                "AllReduce", ALU.add,
                replica_groups=[list(range(NCORE))],
                ins=[sloc[0:2].opt()], outs=[sglob[0:2].opt()],
            )
            prep_ln(2)
            load_mask(3)
            kproj(2)
            scores_block(2, masks.pop(2))
            prep_ln(3)
            loads_after(1)
            loads_after(2)
            kproj(3)
            scores_block(3, masks.pop(3))

            # second half all-reduce (batches 2,3), after b3's sums land
            nc.gpsimd.collective_compute(
                "AllReduce", ALU.add,
                replica_groups=[list(range(NCORE))],
                ins=[sloc[2:4].opt()], outs=[sglob[2:4].opt()],
            )
            rec_raw = sb.tile([NQ, B, H], F32, tag="rec_raw")
            nc.sync.dma_start(
                rec_raw[:, 0:2, :], sglob[0:2].transpose([1, 0, 2]))
            nc.sync.dma_start(
                rec_raw[:, 2:4, :], sglob[2:4].transpose([1, 0, 2]))

            # ---- phase 2: V projection + context, overlapping the collective
            # reverse order so vp[b] can reuse kvnT[b+1]'s memory
            vp = {}
            vp[0] = sb.tile([128, NKT, H, HD], BF16, tag="vpx", name="vp_0")
            for b in (1, 2, 3):
                vp[b] = sb.tile([128, NKT, H, HD], BF16, tag=f"kvnT{b - 1}",
                                name=f"vp_{b}")

            # unnormalized transposed context, bf16 (normalized later)
            ctxTu = sb.tile([128, NDC, B, NQ], BF16, tag="qnT", name="ctxTu")

            def vproj_block(b):
                for r in range(NKT):
                    for nh in range(2):
                        acc = ps.tile([128, 512], F32, tag="mm", bufs=2)
                        for k in range(NDC):
                            nc.tensor.matmul(
                                acc[:],
                                lhsT=kvnT[b][:, k, r, :],
                                rhs=wv_sb[:, k, nh * 512:(nh + 1) * 512],
                                start=(k == 0), stop=(k == NDC - 1),
                            )
                        nc.vector.tensor_copy(
                            out=vp[b][:, r, nh * 8:(nh + 1) * 8, :],
                            in_=acc[:].rearrange("p (g d) -> p g d", g=8),
                        )

            def ctx_block(b):
                for j in range(NDC):
                    ctx_ps = ps.tile([128, NQ], F32, tag="sc", bufs=2)
                    for hh in range(2):
                        h = 2 * j + hh
                        for c in range(NKT):
                            nc.tensor.matmul(
                                ctx_ps[hh * HD:(hh + 1) * HD, :],
                                lhsT=vp[b][:, c, h, :],
                                rhs=exp_all[b][:, c, h, :],
                                start=(c == 0), stop=(c == NKT - 1),
                            )
                    nc.scalar.activation(
                        ctxTu[:, j, b, :], ctx_ps[:], AF.Copy,
                    )

            ctxT = sb.tile([128, NDC, B, NQ], BF16, tag="kpT", name="ctxT")
            out_sb = sb.tile([128, NDC, B, NQ], BF16, tag="wq", bufs=1,
                             name="out_sb")

            def rec_half(half):
                """reciprocal + transpose into rec_bf for batches 2h, 2h+1."""
                b0, b1 = 2 * half, 2 * half + 1
                nc.vector.reciprocal(
                    rec_raw[:, b0:b1 + 1, :], rec_raw[:, b0:b1 + 1, :])
                rec_cast = sb.tile([NQ, 2, H], BF16, tag="rec_cast", bufs=2)
                nc.vector.tensor_copy(
                    out=rec_cast[:], in_=rec_raw[:, b0:b1 + 1, :])
                recT_ps = ps.tile([128, NQ], BF16, tag="tr", bufs=2)
                for i, b in enumerate((b0, b1)):
                    nc.tensor.transpose(
                        recT_ps[32 * (b % 2):32 * (b % 2) + 16, :],
                        rec_cast[:, i, :], ident[:NQ, :NQ],
                    )
                for b in (b0, b1):
                    p0 = 32 * (b % 2)
                    nc.vector.tensor_copy(
                        out=rec_bf[p0:p0 + 16, half, :],
                        in_=recT_ps[p0:p0 + 16, :],
                    )

            def norm_half(half):
                for b in (2 * half, 2 * half + 1):
                    for j in range(NDC):
                        rpp = ps.tile([128, NQ], F32,
                                      tag=("den" if j % 2 else "sc"), bufs=2)
                        nc.tensor.matmul(
                            rpp[:],
                            lhsT=sel_sb[:, b % 2, j, :],
                            rhs=rec_bf[:, b // 2, :],
                            start=True, stop=True,
                        )
                        nc.vector.tensor_mul(
                            ctxT[:, j, b, :], ctxTu[:, j, b, :], rpp[:]
                        )

            def outproj():
                for m in range(NDC):
                    acc = ps.tile([128, 512], F32, tag="mm", bufs=2)
                    for k in range(NDC):
                        nc.tensor.matmul(
                            acc[:, 0:B * NQ],
                            lhsT=wo_sb[:, k, m * 128:(m + 1) * 128],
                            rhs=ctxT[:, k, :, :].rearrange("p b q -> p (b q)"),
                            start=(k == 0), stop=(k == NDC - 1),
                        )
                    nc.vector.tensor_copy(
                        out=out_sb[:, m, :, :],
                        in_=acc[:, 0:B * NQ].rearrange("p (b q) -> p b q", b=B),
                    )
                    nc.sync.dma_start(out_d[:, m], out_sb[:, m])

            # phase 2 with the first-half normalize folded into the middle
            vproj_block(0); ctx_block(0)
            vproj_block(1); ctx_block(1)
            rec_half(0)
            norm_half(0)
            vproj_block(2); ctx_block(2)
            vproj_block(3); ctx_block(3)
            rec_half(1)
            norm_half(1)
            outproj()

    nc.compile()
    return nc


def _prep_in_maps(q, kv, mask, in_proj_w, in_proj_b, out_w, out_b,
                  g_q, b_q, g_kv, b_kv):
    """Host-side prep: fold LN affine + V-bias, shard kv/mask per core.

    Returns (in_maps, bias_total)."""
    q = np.asarray(q, np.float32)
    kv = np.asarray(kv, np.float32)
    mask = np.asarray(mask)
    in_proj_w = np.asarray(in_proj_w, np.float32)
    in_proj_b = np.asarray(in_proj_b, np.float32)
    out_w = np.asarray(out_w, np.float32)
    out_b = np.asarray(out_b, np.float32)
    g_q = np.asarray(g_q, np.float32)
    b_q = np.asarray(b_q, np.float32)
    g_kv = np.asarray(g_kv, np.float32)
    b_kv = np.asarray(b_kv, np.float32)

    Wq, Wk, Wv = in_proj_w[:D], in_proj_w[D:2 * D], in_proj_w[2 * D:]
    bq, bk, bv = in_proj_b[:D], in_proj_b[D:2 * D], in_proj_b[2 * D:]

    # Fold LayerNorm affine into projections: LN(x)*g+b @ W^T + c
    #   = LN(x) @ (W*g)^T + (W@b + c)
    WqT = (Wq * g_q[None, :]).T.astype(ml_dtypes.bfloat16)
    WkT = (Wk * g_kv[None, :]).T.astype(ml_dtypes.bfloat16)
    WvT = (Wv * g_kv[None, :]).T.astype(ml_dtypes.bfloat16)
    bq_eff = (bq + Wq @ b_q).astype(np.float32)
    bk_eff = (bk + Wk @ b_kv).astype(np.float32)
    bv_eff = (bv + Wv @ b_kv).astype(np.float32)
    # V bias passes through softmax unchanged (weights sum to 1): fold into
    # the final output bias on the host.
    WoT = out_w.T.astype(ml_dtypes.bfloat16)
    bias_total = (out_b + out_w @ bv_eff).astype(np.float32)

    # per-query key mask; all-zero mask rows attend everywhere
    kv16 = kv.astype(ml_dtypes.bfloat16)
    allowed = (mask != 0)
    has_any = allowed.any(axis=-1, keepdims=True)
    eff = np.where(has_any, allowed, True)  # [B, NQ, HW] bool

    # selector for the reciprocal partition-broadcast:
    # sel[p, a, j, c] = 1 iff p == 32a + 2j + c//64  (a = b%2)
    sel = np.zeros((128, 2, NDC, 128), dtype=ml_dtypes.bfloat16)
    for a in range(2):
        for j in range(NDC):
            for cb in range(2):
                sel[32 * a + 2 * j + cb, a, j, cb * 64:(cb + 1) * 64] = 1.0

    common = {
        "q": np.ascontiguousarray(q.astype(ml_dtypes.bfloat16)),
        "wqT": np.ascontiguousarray(WqT.reshape(NDC, 128, D).transpose(1, 0, 2)),
        "wkT": np.ascontiguousarray(WkT.reshape(NDC, 128, D).transpose(1, 0, 2)),
        "wvT": np.ascontiguousarray(WvT.reshape(NDC, 128, D).transpose(1, 0, 2)),
        "woT": np.ascontiguousarray(WoT.reshape(NDC, 128, D).transpose(1, 0, 2)),
        "biasq": np.ascontiguousarray(bq_eff.reshape(NDC, 128).T),
        "biask": np.ascontiguousarray(bk_eff.reshape(NDC, 128).T),
        "sel": sel,
        "ident": np.eye(128, dtype=ml_dtypes.bfloat16),
    }
    in_maps = []
    for c in range(NCORE):
        sl = slice(c * KC, (c + 1) * KC)
        kv_c = kv16[:, sl, :].reshape(B, NKT, 128, D)
        # mask slice -> [B, 128, NKT, NQ] bf16 (keysub-tile on partitions)
        m_c = eff[:, :, sl].transpose(0, 2, 1).reshape(B, NKT, 128, NQ)
        m_c = m_c.transpose(0, 2, 1, 3).astype(ml_dtypes.bfloat16)
        in_maps.append({
            **common,
            "kv": np.ascontiguousarray(kv_c),
            "maskT": np.ascontiguousarray(m_c),
        })
    return in_maps, bias_total


def kernel(q, kv, mask, in_proj_w, in_proj_b, out_w, out_b, g_q, b_q, g_kv, b_kv):
    in_maps, bias_total = _prep_in_maps(
        q, kv, mask, in_proj_w, in_proj_b, out_w, out_b, g_q, b_q, g_kv, b_kv
    )
    if "nc" not in _compiled:
        _compiled["nc"] = _build()
    nc = _compiled["nc"]

    res = run_bass_kernel_spmd(nc, in_maps, core_ids=list(range(NCORE)))

    out = np.zeros((B, NQ, D), np.float32)
    for c in range(NCORE):
        part = res.results[c]["out"].astype(np.float32)
        out += part.transpose(2, 3, 1, 0).reshape(B, NQ, D)
    out += bias_total[None, None, :]
    return out


# revision 35
# speedup vs baseline: 1.1277x; 1.1277x over previous
"""Masked cross-attention (B=4, NQ=100, HW=4096, D=1024, H=16) on 8 TRN2 cores.

Sharding: kv rows (keys) are split 8 ways; each core runs LayerNorm + K/V
projection on its 512-key slice per batch, computes unnormalized partial
attention for all (b, h) against its keys, all-reduces the softmax
denominators on device, normalizes, and computes a partial out-projection.
The host sums the 8 partial outputs and adds the folded bias.

v2 schedule: the kernel is split so the denominator all-reduce overlaps
compute. Phase 1 (per batch): kv LN + transpose, K projection, scores,
exp, mask, and the denominator row-sums (matmul against a ones column).
After batch 3's denominators are DMA'd the all-reduce is kicked. Phase 2
(V projection + context matmuls, ~80us of tensor work) runs while the
collective is in flight. The tail normalizes the context by the global
reciprocals (broadcast across partitions via a selector matmul) and runs
the out-projection.
"""
import sys

sys.path.insert(0, "/opt/trn_rl_repo")

import numpy as np
import ml_dtypes

import concourse.bacc as bacc
import concourse.bass as bass
import concourse.mybir as mybir
import concourse.tile as tile
from concourse.bass_utils import run_bass_kernel_spmd
from concourse.masks import make_identity

B, NQ, HW, D, H = 4, 100, 4096, 1024, 16
HD = D // H          # 64
NCORE = 8
KC = HW // NCORE     # 512 keys per core per batch
NKT = KC // 128      # 4 key sub-tiles of 128
NDC = D // 128       # 8 chunks of the model dim
EPS = 1e-5
SCALE = 1.0 / np.sqrt(np.float32(HD))  # 1/8

F32 = mybir.dt.float32
BF16 = mybir.dt.bfloat16
AF = mybir.ActivationFunctionType
ALU = mybir.AluOpType

_compiled = {}


def _build():
    nc = bacc.Bacc("TRN2", target_bir_lowering=False, num_devices=NCORE)

    kv_d = nc.dram_tensor("kv", [B, NKT, 128, D], BF16, kind="ExternalInput")
    q_d = nc.dram_tensor("q", [B, NQ, D], BF16, kind="ExternalInput")
    mask_d = nc.dram_tensor("maskT", [B, 128, NKT, NQ], BF16, kind="ExternalInput")
    wq_d = nc.dram_tensor("wqT", [128, NDC, D], BF16, kind="ExternalInput")
    wk_d = nc.dram_tensor("wkT", [128, NDC, D], BF16, kind="ExternalInput")
    wv_d = nc.dram_tensor("wvT", [128, NDC, D], BF16, kind="ExternalInput")
    wo_d = nc.dram_tensor("woT", [128, NDC, D], BF16, kind="ExternalInput")
    bq_d = nc.dram_tensor("biasq", [128, NDC], F32, kind="ExternalInput")
    bk_d = nc.dram_tensor("biask", [128, NDC], F32, kind="ExternalInput")
    sel_d = nc.dram_tensor("sel", [128, 2, NDC, 128], BF16, kind="ExternalInput")
    ident_d = nc.dram_tensor("ident", [128, 128], BF16, kind="ExternalInput")
    out_d = nc.dram_tensor("out", [128, NDC, B, NQ], BF16, kind="ExternalOutput")

    with tile.TileContext(nc) as tc:
        with (
            tc.tile_pool(name="sb", bufs=1) as sb,
            tc.tile_pool(name="ps", bufs=1, space="PSUM") as ps,
            tc.tile_pool(name="dram", bufs=1, space="DRAM") as dram,
        ):
            # ---- constants ----
            ident = sb.tile([128, 128], BF16, tag="ident")
            nc.sync.dma_start(ident[:], ident_d[:])
            eps_t = sb.tile([128, 1], F32, tag="eps")
            nc.vector.memset(eps_t[:], EPS)
            ones1 = sb.tile([128, 1], BF16, tag="ones1")
            nc.vector.memset(ones1[:], 1.0)
            # reciprocal-broadcast rhs: partition 32*(b%2)+h, free (b//2, q);
            # unused partition rows stay 1.0 (sel zeroes their contribution;
            # avoids 0*NaN in the broadcast matmul)
            rec_bf = sb.tile([128, 2, NQ], BF16, tag="rec_in", name="rec_bf")
            nc.vector.memset(rec_bf[:], 1.0)

            # ---- weights ----
            wk_sb = sb.tile([128, NDC, D], BF16, tag="wk")
            wv_sb = sb.tile([128, NDC, D], BF16, tag="wv")
            wq_sb = sb.tile([128, NDC, D], BF16, tag="wq", bufs=1, name="wq")
            wo_sb = sb.tile([128, NDC, D], BF16, tag="wo")
            bqv_sb = sb.tile([128, NDC], F32, tag="bqv")
            bkv_sb = sb.tile([128, NDC], F32, tag="bkv")
            bq_sb = [bqv_sb[:, j:j + 1] for j in range(NDC)]
            bk_sb = [bkv_sb[:, j:j + 1] for j in range(NDC)]
            sel_sb = sb.tile([128, 2, NDC, 128], BF16, tag="sel")

            # persistent activations
            kvnT = [
                sb.tile([128, NDC, NKT, 128], BF16, tag=f"kvnT{b}",
                        name=f"kvnT_{b}")
                for b in range(B)
            ]
            exp_all = [
                sb.tile([128, NKT, H, NQ], BF16, tag=f"exp{b}", name=f"exp_{b}")
                for b in range(B)
            ]
            qnT = sb.tile([128, NDC, B, NQ], BF16, tag="qnT", name="qnT")
            qpT = sb.tile([128, NDC, B, 2, NQ], BF16, tag="qpT", name="qpT")
            kpT = sb.tile([128, NDC, KC], BF16, tag="kpT", name="kpT0")

            den_all = sb.tile([NQ, B, H], F32, tag="den_all")
            sloc = dram.tile([B, NQ, H], F32)
            sglob = dram.tile([B, NQ, H], F32, tag="sglob")

            def layernorm_to_bf16(x, xn_bf16, p):
                """(x - mean) * rsqrt(var + eps), row-wise over the free dim."""
                stats = sb.tile([128, 2, 6], F32, tag="lnstats", bufs=4)
                nc.vector.bn_stats(stats[:p, 0, :], x[:p, 0:512])
                nc.vector.bn_stats(stats[:p, 1, :], x[:p, 512:1024])
                mv = sb.tile([128, 2], F32, tag="lnmv", bufs=4)
                nc.vector.bn_aggr(mv[:p], stats[:p])
                rstd = sb.tile([128, 1], F32, tag="lnrstd", bufs=4)
                nc.scalar.activation(rstd[:p], mv[:p, 1:2], AF.Sqrt, bias=eps_t[:p])
                nc.vector.reciprocal(rstd[:p], rstd[:p])
                nc.vector.tensor_scalar(
                    xn_bf16[:p], x[:p], mv[:p, 0:1], rstd[:p],
                    ALU.subtract, ALU.mult,
                )

            def prep_ln(b):
                """kv load + LN + transpose for batch b."""
                for r in range(NKT):
                    kvraw = sb.tile([128, D], BF16, tag="kvraw", bufs=4)
                    nc.sync.dma_start(kvraw[:], kv_d[b, r])
                    xn = sb.tile([128, D], BF16, tag="xn", bufs=2)
                    layernorm_to_bf16(kvraw, xn, 128)
                    for k4 in range(NDC // 4):
                        tr = ps.tile([128, 4, 128], BF16, tag="tr", bufs=2)
                        for kk in range(4):
                            k = 4 * k4 + kk
                            nc.tensor.transpose(
                                tr[:, kk, :], xn[:, k * 128:(k + 1) * 128], ident[:]
                            )
                        nc.vector.tensor_copy(
                            out=kvnT[b][:, 4 * k4:4 * k4 + 4, r, :], in_=tr[:]
                        )

            def kproj(b):
                # K projection -> kpT[:, j, :]: [128 dout, KC keys] (+bias)
                for j in range(NDC):
                    acc = ps.tile([128, KC], F32, tag="mm", bufs=2)
                    for k in range(NDC):
                        nc.tensor.matmul(
                            acc[:],
                            lhsT=wk_sb[:, k, j * 128:(j + 1) * 128],
                            rhs=kvnT[b][:, k, :, :].rearrange("p r c -> p (r c)"),
                            start=(k == 0), stop=(k == NDC - 1),
                        )
                    nc.scalar.activation(
                        kpT[:, j, :], acc[:], AF.Identity, bias=bk_sb[j][:]
                    )

            def q_pipeline():
                """LayerNorm + transpose + projection of q, all batches."""
                for b in range(B):
                    qraw = sb.tile([NQ, D], BF16, tag="kvraw", bufs=4)
                    nc.sync.dma_start(qraw[:], q_d[b])
                    qn = sb.tile([NQ, D], BF16, tag="xn", bufs=2)
                    layernorm_to_bf16(qraw, qn, NQ)
                    for k4 in range(NDC // 4):
                        tr = ps.tile([128, 4, NQ], BF16, tag="tr", bufs=2)
                        for kk in range(4):
                            k = 4 * k4 + kk
                            nc.tensor.transpose(
                                tr[:, kk, :], qn[:, k * 128:(k + 1) * 128],
                                ident[:NQ, :NQ],
                            )
                        nc.vector.tensor_copy(
                            out=qnT[:, 4 * k4:4 * k4 + 4, b, :], in_=tr[:]
                        )
                # qpT[:, j]: [128, B, 2, NQ] block-diagonal by head: rows 0:64
                # hold head 2j over i=0 columns, rows 64:128 hold head 2j+1
                # over i=1 columns, zeros elsewhere.
                nc.vector.memset(qpT[:], 0.0)
                for j in range(NDC):
                    acc = ps.tile([128, 512], F32, tag="mm", bufs=2)
                    for k in range(NDC):
                        nc.tensor.matmul(
                            acc[:, 0:B * NQ],
                            lhsT=wq_sb[:, k, j * 128:(j + 1) * 128],
                            rhs=qnT[:, k, :, :].rearrange("p b q -> p (b q)"),
                            start=(k == 0), stop=(k == NDC - 1),
                        )
                    nc.scalar.activation(
                        qpT[0:HD, j, :, 0, :],
                        acc[0:HD, 0:B * NQ].rearrange("p (b q) -> p b q", b=B),
                        AF.Identity, bias=bq_sb[j][0:HD],
                    )
                    nc.scalar.activation(
                        qpT[HD:128, j, :, 1, :],
                        acc[HD:128, 0:B * NQ].rearrange("p (b q) -> p b q", b=B),
                        AF.Identity, bias=bq_sb[j][HD:128],
                    )

            def scores_block(b, mask_b):
                """scores^T, exp, mask, denominator row-sums for batch b."""
                for j in range(NDC):
                    for c2 in range(2):
                        sc = ps.tile([128, 2, 2, NQ], F32, tag="sc", bufs=2)
                        for cc in range(2):
                            c = 2 * c2 + cc
                            nc.tensor.matmul(
                                sc[:, cc, :, :].rearrange("p i q -> p (i q)"),
                                lhsT=kpT[:, j, c * 128:(c + 1) * 128],
                                rhs=qpT[:, j, b, :, :].rearrange(
                                    "p i q -> p (i q)"),
                                start=True, stop=True,
                            )
                        nc.scalar.activation(
                            exp_all[b][:, 2 * c2:2 * c2 + 2, 2 * j:2 * j + 2, :],
                            sc[:], AF.Exp, scale=float(SCALE),
                        )
                    for hh in range(2):
                        h = 2 * j + hh
                        nc.vector.tensor_mul(
                            exp_all[b][:, :, h, :], exp_all[b][:, :, h, :],
                            mask_b[:],
                        )
                    if j % 2 == 1:
                        # heads 4g..4g+3 masked: pre-sum the 4 key-subtiles
                        # on DVE, then one [NQ,1] matmul per head
                        g = j // 2
                        s01 = sb.tile([128, 4, NQ], BF16, tag="csum", bufs=4)
                        nc.vector.tensor_tensor(
                            out=s01[:], in0=exp_all[b][:, 0, 4 * g:4 * g + 4, :],
                            in1=exp_all[b][:, 1, 4 * g:4 * g + 4, :],
                            op=ALU.add,
                        )
                        s23 = sb.tile([128, 4, NQ], BF16, tag="csum", bufs=4)
                        nc.vector.tensor_tensor(
                            out=s23[:], in0=exp_all[b][:, 2, 4 * g:4 * g + 4, :],
                            in1=exp_all[b][:, 3, 4 * g:4 * g + 4, :],
                            op=ALU.add,
                        )
                        s = sb.tile([128, 4, NQ], BF16, tag="csum", bufs=4)
                        nc.vector.tensor_tensor(
                            out=s[:], in0=s01[:], in1=s23[:], op=ALU.add,
                        )
                        for i in range(4):
                            den_q = ps.tile([NQ, 1], F32, tag="den", bufs=2)
                            nc.tensor.matmul(
                                den_q[:], lhsT=s[:, i, :], rhs=ones1[:],
                                start=True, stop=True,
                            )
                            nc.scalar.activation(
                                den_all[:, b, 4 * g + i:4 * g + i + 1],
                                den_q[:], AF.Copy,
                            )
                nc.sync.dma_start(sloc[b], den_all[:, b, :])

            # ---- phase 1: everything the denominators need ----
            mask0 = sb.tile([128, NKT, NQ], BF16, tag="maskb", bufs=2)
            nc.sync.dma_start(mask0[:], mask_d[0])
            masks = {0: mask0}

            def loads_after(b):
                # staggered weight loads on the sync queue, after each
                # batch's kv so the critical kv tiles lead
                if b == 0:
                    nc.sync.dma_start(wk_sb[:], wk_d[:])
                    nc.sync.dma_start(bkv_sb[:], bk_d[:])
                    nc.sync.dma_start(wq_sb[:], wq_d[:])
                    nc.sync.dma_start(bqv_sb[:], bq_d[:])
                elif b == 1:
                    nc.sync.dma_start(wv_sb[:], wv_d[:])
                elif b == 2:
                    nc.sync.dma_start(wo_sb[:], wo_d[:])
                    nc.sync.dma_start(sel_sb[:], sel_d[:])

            def load_mask(b):
                m = sb.tile([128, NKT, NQ], BF16, tag="maskb", bufs=2)
                nc.sync.dma_start(m[:], mask_d[b])
                masks[b] = m

            prep_ln(0)
            prep_ln(1)          # kv1 rides the ring before the weights so
            loads_after(0)      # its transposes cover the wk wait
            kproj(0)
            load_mask(1)
            q_pipeline()
            scores_block(0, masks.pop(0))
            kproj(1)
            load_mask(2)
            scores_block(1, masks.pop(1))
            # first half all-reduce (batches 0,1) overlaps phase 1
            nc.gpsimd.collective_compute(
                "AllReduce", ALU.add,
                replica_groups=[list(range(NCORE))],
                ins=[sloc[0:2].opt()], outs=[sglob[0:2].opt()],
            )
            prep_ln(2)
            load_mask(3)
            kproj(2)
            scores_block(2, masks.pop(2))
            prep_ln(3)
            loads_after(1)
            loads_after(2)
            kproj(3)
            scores_block(3, masks.pop(3))

            # second half all-reduce (batches 2,3), after b3's sums land
            nc.gpsimd.collective_compute(
                "AllReduce", ALU.add,
                replica_groups=[list(range(NCORE))],
                ins=[sloc[2:4].opt()], outs=[sglob[2:4].opt()],
            )
            rec_raw = sb.tile([NQ, B, H], F32, tag="rec_raw")
            nc.sync.dma_start(
                rec_raw[:, 0:2, :], sglob[0:2].transpose([1, 0, 2]))
            nc.sync.dma_start(
                rec_raw[:, 2:4, :], sglob[2:4].transpose([1, 0, 2]))

            # ---- phase 2: V projection + context, overlapping the collective
            # reverse order so vp[b] can reuse kvnT[b+1]'s memory
            vp = {}
            vp[0] = sb.tile([128, NKT, H, HD], BF16, tag="vpx", name="vp_0")
            for b in (1, 2, 3):
                vp[b] = sb.tile([128, NKT, H, HD], BF16, tag=f"kvnT{b - 1}",
                                name=f"vp_{b}")

            # unnormalized transposed context, bf16 (normalized later)
            ctxTu = sb.tile([128, NDC, B, NQ], BF16, tag="qnT", name="ctxTu")

            def vproj_block(b):
                for r in range(NKT):
                    for nh in range(2):
                        acc = ps.tile([128, 512], F32, tag="mm", bufs=2)
                        for k in range(NDC):
                            nc.tensor.matmul(
                                acc[:],
                                lhsT=kvnT[b][:, k, r, :],
                                rhs=wv_sb[:, k, nh * 512:(nh + 1) * 512],
                                start=(k == 0), stop=(k == NDC - 1),
                            )
                        nc.vector.tensor_copy(
                            out=vp[b][:, r, nh * 8:(nh + 1) * 8, :],
                            in_=acc[:].rearrange("p (g d) -> p g d", g=8),
                        )

            def ctx_block(b):
                for j in range(NDC):
                    ctx_ps = ps.tile([128, NQ], F32, tag="sc", bufs=2)
                    for hh in range(2):
                        h = 2 * j + hh
                        for c in range(NKT):
                            nc.tensor.matmul(
                                ctx_ps[hh * HD:(hh + 1) * HD, :],
                                lhsT=vp[b][:, c, h, :],
                                rhs=exp_all[b][:, c, h, :],
                                start=(c == 0), stop=(c == NKT - 1),
                            )
                    nc.scalar.activation(
                        ctxTu[:, j, b, :], ctx_ps[:], AF.Copy,
                    )

            ctxT = sb.tile([128, NDC, B, NQ], BF16, tag="kpT", name="ctxT")
            out_sb = sb.tile([128, NDC, B, NQ], BF16, tag="wq", bufs=1,
                             name="out_sb")

            def rec_half(half):
                """reciprocal + transpose into rec_bf for batches 2h, 2h+1."""
                b0, b1 = 2 * half, 2 * half + 1
                nc.vector.reciprocal(
                    rec_raw[:, b0:b1 + 1, :], rec_raw[:, b0:b1 + 1, :])
                rec_cast = sb.tile([NQ, 2, H], BF16, tag="rec_cast", bufs=2)
                nc.vector.tensor_copy(
                    out=rec_cast[:], in_=rec_raw[:, b0:b1 + 1, :])
                recT_ps = ps.tile([128, NQ], BF16, tag="tr", bufs=2)
                for i, b in enumerate((b0, b1)):
                    nc.tensor.transpose(
                        recT_ps[32 * (b % 2):32 * (b % 2) + 16, :],
                        rec_cast[:, i, :], ident[:NQ, :NQ],
                    )
                for b in (b0, b1):
                    p0 = 32 * (b % 2)
                    nc.vector.tensor_copy(
                        out=rec_bf[p0:p0 + 16, half, :],
                        in_=recT_ps[p0:p0 + 16, :],
                    )

            def norm_half(half):
                for b in (2 * half, 2 * half + 1):
                    for j in range(NDC):
                        rpp = ps.tile([128, NQ], F32,
                                      tag=("den" if j % 2 else "sc"), bufs=2)
                        nc.tensor.matmul(
                            rpp[:],
                            lhsT=sel_sb[:, b % 2, j, :],
                            rhs=rec_bf[:, b // 2, :],
                            start=True, stop=True,
                        )
                        nc.vector.tensor_mul(
                            ctxT[:, j, b, :], ctxTu[:, j, b, :], rpp[:]
                        )

            def outproj():
                for m in range(NDC):
                    acc = ps.tile([128, 512], F32, tag="mm", bufs=2)
                    for k in range(NDC):
                        nc.tensor.matmul(
                            acc[:, 0:B * NQ],
                            lhsT=wo_sb[:, k, m * 128:(m + 1) * 128],
                            rhs=ctxT[:, k, :, :].rearrange("p b q -> p (b q)"),
                            start=(k == 0), stop=(k == NDC - 1),
                        )
                    nc.vector.tensor_copy(
                        out=out_sb[:, m, :, :],
                        in_=acc[:, 0:B * NQ].rearrange("p (b q) -> p b q", b=B),
                    )
                    nc.sync.dma_start(out_d[:, m], out_sb[:, m])

            # phase 2 with the first-half normalize folded into the middle
            vproj_block(0); ctx_block(0)
            vproj_block(1); ctx_block(1)
            rec_half(0)
            norm_half(0)
            vproj_block(2); ctx_block(2)
            vproj_block(3); ctx_block(3)
            rec_half(1)
            norm_half(1)
            outproj()

    nc.compile()
    return nc


def _prep_in_maps(q, kv, mask, in_proj_w, in_proj_b, out_w, out_b,
                  g_q, b_q, g_kv, b_kv):
    """Host-side prep: fold LN affine + V-bias, shard kv/mask per core.

    Returns (in_maps, bias_total)."""
    q = np.asarray(q, np.float32)
    kv = np.asarray(kv, np.float32)
    mask = np.asarray(mask)
    in_proj_w = np.asarray(in_proj_w, np.float32)
    in_proj_b = np.asarray(in_proj_b, np.float32)
    out_w = np.asarray(out_w, np.float32)
    out_b = np.asarray(out_b, np.float32)
    g_q = np.asarray(g_q, np.float32)
    b_q = np.asarray(b_q, np.float32)
    g_kv = np.asarray(g_kv, np.float32)
    b_kv = np.asarray(b_kv, np.float32)

    Wq, Wk, Wv = in_proj_w[:D], in_proj_w[D:2 * D], in_proj_w[2 * D:]
    bq, bk, bv = in_proj_b[:D], in_proj_b[D:2 * D], in_proj_b[2 * D:]

    # Fold LayerNorm affine into projections: LN(x)*g+b @ W^T + c
    #   = LN(x) @ (W*g)^T + (W@b + c)
    WqT = (Wq * g_q[None, :]).T.astype(ml_dtypes.bfloat16)
    WkT = (Wk * g_kv[None, :]).T.astype(ml_dtypes.bfloat16)
    WvT = (Wv * g_kv[None, :]).T.astype(ml_dtypes.bfloat16)
    bq_eff = (bq + Wq @ b_q).astype(np.float32)
    bk_eff = (bk + Wk @ b_kv).astype(np.float32)
    bv_eff = (bv + Wv @ b_kv).astype(np.float32)
    # V bias passes through softmax unchanged (weights sum to 1): fold into
    # the final output bias on the host.
    WoT = out_w.T.astype(ml_dtypes.bfloat16)
    bias_total = (out_b + out_w @ bv_eff).astype(np.float32)

    # per-query key mask; all-zero mask rows attend everywhere
    kv16 = kv.astype(ml_dtypes.bfloat16)
    allowed = (mask != 0)
    has_any = allowed.any(axis=-1, keepdims=True)
    eff = np.where(has_any, allowed, True)  # [B, NQ, HW] bool

    # selector for the reciprocal partition-broadcast:
    # sel[p, a, j, c] = 1 iff p == 32a + 2j + c//64  (a = b%2)
    sel = np.zeros((128, 2, NDC, 128), dtype=ml_dtypes.bfloat16)
    for a in range(2):
        for j in range(NDC):
            for cb in range(2):
                sel[32 * a + 2 * j + cb, a, j, cb * 64:(cb + 1) * 64] = 1.0

    common = {
        "q": np.ascontiguousarray(q.astype(ml_dtypes.bfloat16)),
        "wqT": np.ascontiguousarray(WqT.reshape(NDC, 128, D).transpose(1, 0, 2)),
        "wkT": np.ascontiguousarray(WkT.reshape(NDC, 128, D).transpose(1, 0, 2)),
        "wvT": np.ascontiguousarray(WvT.reshape(NDC, 128, D).transpose(1, 0, 2)),
        "woT": np.ascontiguousarray(WoT.reshape(NDC, 128, D).transpose(1, 0, 2)),
        "biasq": np.ascontiguousarray(bq_eff.reshape(NDC, 128).T),
        "biask": np.ascontiguousarray(bk_eff.reshape(NDC, 128).T),
        "sel": sel,
        "ident": np.eye(128, dtype=ml_dtypes.bfloat16),
    }
    in_maps = []
    for c in range(NCORE):
        sl = slice(c * KC, (c + 1) * KC)
        kv_c = kv16[:, sl, :].reshape(B, NKT, 128, D)
        # mask slice -> [B, 128, NKT, NQ] bf16 (keysub-tile on partitions)
        m_c = eff[:, :, sl].transpose(0, 2, 1).reshape(B, NKT, 128, NQ)
        m_c = m_c.transpose(0, 2, 1, 3).astype(ml_dtypes.bfloat16)
        in_maps.append({
            **common,
            "kv": np.ascontiguousarray(kv_c),
            "maskT": np.ascontiguousarray(m_c),
        })
    return in_maps, bias_total


def kernel(q, kv, mask, in_proj_w, in_proj_b, out_w, out_b, g_q, b_q, g_kv, b_kv):
    in_maps, bias_total = _prep_in_maps(
        q, kv, mask, in_proj_w, in_proj_b, out_w, out_b, g_q, b_q, g_kv, b_kv
    )
    if "nc" not in _compiled:
        _compiled["nc"] = _build()
    nc = _compiled["nc"]

    res = run_bass_kernel_spmd(nc, in_maps, core_ids=list(range(NCORE)))

    out = np.zeros((B, NQ, D), np.float32)
    for c in range(NCORE):
        part = res.results[c]["out"].astype(np.float32)
        out += part.transpose(2, 3, 1, 0).reshape(B, NQ, D)
    out += bias_total[None, None, :]
    return out


# revision 36
# speedup vs baseline: 1.1473x; 1.0174x over previous
"""Masked cross-attention (B=4, NQ=100, HW=4096, D=1024, H=16) on 8 TRN2 cores.

Sharding: kv rows (keys) are split 8 ways; each core runs LayerNorm + K/V
projection on its 512-key slice per batch, computes unnormalized partial
attention for all (b, h) against its keys, all-reduces the softmax
denominators on device, normalizes, and computes a partial out-projection.
The host sums the 8 partial outputs and adds the folded bias.

v2 schedule: the kernel is split so the denominator all-reduce overlaps
compute. Phase 1 (per batch): kv LN + transpose, K projection, scores,
exp, mask, and the denominator row-sums (matmul against a ones column).
After batch 3's denominators are DMA'd the all-reduce is kicked. Phase 2
(V projection + context matmuls, ~80us of tensor work) runs while the
collective is in flight. The tail normalizes the context by the global
reciprocals (broadcast across partitions via a selector matmul) and runs
the out-projection.
"""
import sys

sys.path.insert(0, "/opt/trn_rl_repo")

import numpy as np
import ml_dtypes

import concourse.bacc as bacc
import concourse.bass as bass
import concourse.mybir as mybir
import concourse.tile as tile
from concourse.bass_utils import run_bass_kernel_spmd
from concourse.masks import make_identity

B, NQ, HW, D, H = 4, 100, 4096, 1024, 16
HD = D // H          # 64
NCORE = 8
KC = HW // NCORE     # 512 keys per core per batch
NKT = KC // 128      # 4 key sub-tiles of 128
NDC = D // 128       # 8 chunks of the model dim
EPS = 1e-5
SCALE = 1.0 / np.sqrt(np.float32(HD))  # 1/8

F32 = mybir.dt.float32
BF16 = mybir.dt.bfloat16
AF = mybir.ActivationFunctionType
ALU = mybir.AluOpType

_compiled = {}


def _build():
    nc = bacc.Bacc("TRN2", target_bir_lowering=False, num_devices=NCORE)

    kv_d = nc.dram_tensor("kv", [B, NKT, 128, D], BF16, kind="ExternalInput")
    q_d = nc.dram_tensor("q", [B, NQ, D], BF16, kind="ExternalInput")
    mask_d = nc.dram_tensor("maskT", [B, 128, NKT, NQ], BF16, kind="ExternalInput")
    wq_d = nc.dram_tensor("wqT", [128, NDC, D], BF16, kind="ExternalInput")
    wk_d = nc.dram_tensor("wkT", [128, NDC, D], BF16, kind="ExternalInput")
    wv_d = nc.dram_tensor("wvT", [128, NDC, D], BF16, kind="ExternalInput")
    wo_d = nc.dram_tensor("woT", [128, NDC, D], BF16, kind="ExternalInput")
    bq_d = nc.dram_tensor("biasq", [128, NDC], F32, kind="ExternalInput")
    bk_d = nc.dram_tensor("biask", [128, NDC], F32, kind="ExternalInput")
    sel_d = nc.dram_tensor("sel", [128, 2, NDC, 128], BF16, kind="ExternalInput")
    ident_d = nc.dram_tensor("ident", [128, 128], BF16, kind="ExternalInput")
    out_d = nc.dram_tensor("out", [128, NDC, B, NQ], BF16, kind="ExternalOutput")

    with tile.TileContext(nc) as tc:
        with (
            tc.tile_pool(name="sb", bufs=1) as sb,
            tc.tile_pool(name="ps", bufs=1, space="PSUM") as ps,
            tc.tile_pool(name="dram", bufs=1, space="DRAM") as dram,
        ):
            # ---- constants ----
            ident = sb.tile([128, 128], BF16, tag="ident")
            nc.sync.dma_start(ident[:], ident_d[:])
            eps_t = sb.tile([128, 1], F32, tag="eps")
            nc.vector.memset(eps_t[:], EPS)
            ones1 = sb.tile([128, 1], BF16, tag="ones1")
            nc.vector.memset(ones1[:], 1.0)
            # reciprocal-broadcast rhs: partition 32*(b%2)+h, free (b//2, q);
            # unused partition rows stay 1.0 (sel zeroes their contribution;
            # avoids 0*NaN in the broadcast matmul)
            rec_bf = sb.tile([128, 2, NQ], BF16, tag="rec_in", name="rec_bf")
            nc.vector.memset(rec_bf[:], 1.0)

            # ---- weights ----
            wk_sb = sb.tile([128, NDC, D], BF16, tag="wk")
            wv_sb = sb.tile([128, NDC, D], BF16, tag="wv")
            wq_sb = sb.tile([128, NDC, D], BF16, tag="wq", bufs=1, name="wq")
            wo_sb = sb.tile([128, NDC, D], BF16, tag="wo")
            bqv_sb = sb.tile([128, NDC], F32, tag="bqv")
            bkv_sb = sb.tile([128, NDC], F32, tag="bkv")
            bq_sb = [bqv_sb[:, j:j + 1] for j in range(NDC)]
            bk_sb = [bkv_sb[:, j:j + 1] for j in range(NDC)]
            sel_sb = sb.tile([128, 2, NDC, 128], BF16, tag="sel")

            # persistent activations
            kvnT = [
                sb.tile([128, NDC, NKT, 128], BF16, tag=f"kvnT{b}",
                        name=f"kvnT_{b}")
                for b in range(B)
            ]
            exp_all = [
                sb.tile([128, NKT, H, NQ], BF16, tag=f"exp{b}", name=f"exp_{b}")
                for b in range(B)
            ]
            qnT = sb.tile([128, NDC, B, NQ], BF16, tag="qnT", name="qnT")
            qpT = sb.tile([128, NDC, B, 2, NQ], BF16, tag="qpT", name="qpT")
            kpT = sb.tile([128, NDC, KC], BF16, tag="kpT", name="kpT0")

            den_all = sb.tile([NQ, B, H], F32, tag="den_all")
            sloc = dram.tile([B, NQ, H], F32)
            sglob = dram.tile([B, NQ, H], F32, tag="sglob")

            def layernorm_to_bf16(x, xn_bf16, p):
                """(x - mean) * rsqrt(var + eps), row-wise over the free dim."""
                stats = sb.tile([128, 2, 6], F32, tag="lnstats", bufs=4)
                nc.vector.bn_stats(stats[:p, 0, :], x[:p, 0:512])
                nc.vector.bn_stats(stats[:p, 1, :], x[:p, 512:1024])
                mv = sb.tile([128, 2], F32, tag="lnmv", bufs=4)
                nc.vector.bn_aggr(mv[:p], stats[:p])
                rstd = sb.tile([128, 1], F32, tag="lnrstd", bufs=4)
                nc.scalar.activation(rstd[:p], mv[:p, 1:2], AF.Sqrt, bias=eps_t[:p])
                nc.vector.reciprocal(rstd[:p], rstd[:p])
                nc.vector.tensor_scalar(
                    xn_bf16[:p], x[:p], mv[:p, 0:1], rstd[:p],
                    ALU.subtract, ALU.mult,
                )

            def prep_ln(b):
                """kv load + LN + transpose for batch b."""
                for r in range(NKT):
                    kvraw = sb.tile([128, D], BF16, tag="kvraw", bufs=4)
                    nc.sync.dma_start(kvraw[:], kv_d[b, r])
                    xn = sb.tile([128, D], BF16, tag="xn", bufs=2)
                    layernorm_to_bf16(kvraw, xn, 128)
                    for k4 in range(NDC // 4):
                        tr = ps.tile([128, 4, 128], BF16, tag="tr", bufs=2)
                        for kk in range(4):
                            k = 4 * k4 + kk
                            nc.tensor.transpose(
                                tr[:, kk, :], xn[:, k * 128:(k + 1) * 128], ident[:]
                            )
                        nc.vector.tensor_copy(
                            out=kvnT[b][:, 4 * k4:4 * k4 + 4, r, :], in_=tr[:]
                        )

            def kproj(b):
                # K projection -> kpT[:, j, :]: [128 dout, KC keys] (+bias)
                for j in range(NDC):
                    acc = ps.tile([128, KC], F32, tag="mm", bufs=2)
                    for k in range(NDC):
                        nc.tensor.matmul(
                            acc[:],
                            lhsT=wk_sb[:, k, j * 128:(j + 1) * 128],
                            rhs=kvnT[b][:, k, :, :].rearrange("p r c -> p (r c)"),
                            start=(k == 0), stop=(k == NDC - 1),
                        )
                    nc.scalar.activation(
                        kpT[:, j, :], acc[:], AF.Identity, bias=bk_sb[j][:]
                    )

            def q_pipeline():
                """LayerNorm + transpose + projection of q, all batches."""
                for b in range(B):
                    qraw = sb.tile([NQ, D], BF16, tag="kvraw", bufs=4)
                    nc.sync.dma_start(qraw[:], q_d[b])
                    qn = sb.tile([NQ, D], BF16, tag="xn", bufs=2)
                    layernorm_to_bf16(qraw, qn, NQ)
                    for k4 in range(NDC // 4):
                        tr = ps.tile([128, 4, NQ], BF16, tag="tr", bufs=2)
                        for kk in range(4):
                            k = 4 * k4 + kk
                            nc.tensor.transpose(
                                tr[:, kk, :], qn[:, k * 128:(k + 1) * 128],
                                ident[:NQ, :NQ],
                            )
                        nc.vector.tensor_copy(
                            out=qnT[:, 4 * k4:4 * k4 + 4, b, :], in_=tr[:]
                        )
                # qpT[:, j]: [128, B, 2, NQ] block-diagonal by head: rows 0:64
                # hold head 2j over i=0 columns, rows 64:128 hold head 2j+1
                # over i=1 columns, zeros elsewhere. The zero-fill reads wq so
                # the scheduler cannot hoist it ahead of the kv LN chain.
                for j in range(NDC):
                    nc.vector.tensor_scalar_mul(
                        qpT[:, j, :, :, :].rearrange("p b i q -> p (b i q)"),
                        wq_sb[:, j, 0:B * 2 * NQ], 0.0,
                    )
                for j in range(NDC):
                    acc = ps.tile([128, 512], F32, tag="mm", bufs=2)
                    for k in range(NDC):
                        nc.tensor.matmul(
                            acc[:, 0:B * NQ],
                            lhsT=wq_sb[:, k, j * 128:(j + 1) * 128],
                            rhs=qnT[:, k, :, :].rearrange("p b q -> p (b q)"),
                            start=(k == 0), stop=(k == NDC - 1),
                        )
                    nc.scalar.activation(
                        qpT[0:HD, j, :, 0, :],
                        acc[0:HD, 0:B * NQ].rearrange("p (b q) -> p b q", b=B),
                        AF.Identity, bias=bq_sb[j][0:HD],
                    )
                    nc.scalar.activation(
                        qpT[HD:128, j, :, 1, :],
                        acc[HD:128, 0:B * NQ].rearrange("p (b q) -> p b q", b=B),
                        AF.Identity, bias=bq_sb[j][HD:128],
                    )

            def scores_block(b, mask_b):
                """scores^T, exp, mask, denominator row-sums for batch b."""
                for j in range(NDC):
                    for c2 in range(2):
                        sc = ps.tile([128, 2, 2, NQ], F32, tag="sc", bufs=2)
                        for cc in range(2):
                            c = 2 * c2 + cc
                            nc.tensor.matmul(
                                sc[:, cc, :, :].rearrange("p i q -> p (i q)"),
                                lhsT=kpT[:, j, c * 128:(c + 1) * 128],
                                rhs=qpT[:, j, b, :, :].rearrange(
                                    "p i q -> p (i q)"),
                                start=True, stop=True,
                            )
                        nc.scalar.activation(
                            exp_all[b][:, 2 * c2:2 * c2 + 2, 2 * j:2 * j + 2, :],
                            sc[:], AF.Exp, scale=float(SCALE),
                        )
                    for hh in range(2):
                        h = 2 * j + hh
                        nc.vector.tensor_mul(
                            exp_all[b][:, :, h, :], exp_all[b][:, :, h, :],
                            mask_b[:],
                        )
                    if j % 2 == 1:
                        # heads 4g..4g+3 masked: pre-sum the 4 key-subtiles
                        # on DVE, then one [NQ,1] matmul per head
                        g = j // 2
                        s01 = sb.tile([128, 4, NQ], BF16, tag="csum", bufs=4)
                        nc.vector.tensor_tensor(
                            out=s01[:], in0=exp_all[b][:, 0, 4 * g:4 * g + 4, :],
                            in1=exp_all[b][:, 1, 4 * g:4 * g + 4, :],
                            op=ALU.add,
                        )
                        s23 = sb.tile([128, 4, NQ], BF16, tag="csum", bufs=4)
                        nc.vector.tensor_tensor(
                            out=s23[:], in0=exp_all[b][:, 2, 4 * g:4 * g + 4, :],
                            in1=exp_all[b][:, 3, 4 * g:4 * g + 4, :],
                            op=ALU.add,
                        )
                        s = sb.tile([128, 4, NQ], BF16, tag="csum", bufs=4)
                        nc.vector.tensor_tensor(
                            out=s[:], in0=s01[:], in1=s23[:], op=ALU.add,
                        )
                        for i in range(4):
                            den_q = ps.tile([NQ, 1], F32, tag="den", bufs=2)
                            nc.tensor.matmul(
                                den_q[:], lhsT=s[:, i, :], rhs=ones1[:],
                                start=True, stop=True,
                            )
                            nc.scalar.activation(
                                den_all[:, b, 4 * g + i:4 * g + i + 1],
                                den_q[:], AF.Copy,
                            )
                nc.sync.dma_start(sloc[b], den_all[:, b, :])

            # ---- phase 1: everything the denominators need ----
            mask0 = sb.tile([128, NKT, NQ], BF16, tag="maskb", bufs=2)
            nc.sync.dma_start(mask0[:], mask_d[0])
            masks = {0: mask0}

            def loads_after(b):
                # staggered weight loads on the sync queue, after each
                # batch's kv so the critical kv tiles lead
                if b == 0:
                    nc.sync.dma_start(wk_sb[:], wk_d[:])
                    nc.sync.dma_start(bkv_sb[:], bk_d[:])
                    nc.sync.dma_start(wq_sb[:], wq_d[:])
                    nc.sync.dma_start(bqv_sb[:], bq_d[:])
                elif b == 1:
                    nc.sync.dma_start(wv_sb[:], wv_d[:])
                elif b == 2:
                    nc.sync.dma_start(wo_sb[:], wo_d[:])
                    nc.sync.dma_start(sel_sb[:], sel_d[:])

            def load_mask(b):
                m = sb.tile([128, NKT, NQ], BF16, tag="maskb", bufs=2)
                nc.sync.dma_start(m[:], mask_d[b])
                masks[b] = m

            prep_ln(0)
            prep_ln(1)          # kv1 rides the ring before the weights so
            loads_after(0)      # its transposes cover the wk wait
            kproj(0)
            load_mask(1)
            q_pipeline()
            scores_block(0, masks.pop(0))
            kproj(1)
            load_mask(2)
            scores_block(1, masks.pop(1))
            # first half all-reduce (batches 0,1) overlaps phase 1
            nc.gpsimd.collective_compute(
                "AllReduce", ALU.add,
                replica_groups=[list(range(NCORE))],
                ins=[sloc[0:2].opt()], outs=[sglob[0:2].opt()],
            )
            prep_ln(2)
            load_mask(3)
            kproj(2)
            scores_block(2, masks.pop(2))
            prep_ln(3)
            loads_after(1)
            loads_after(2)
            kproj(3)
            scores_block(3, masks.pop(3))

            # second half all-reduce (batches 2,3), after b3's sums land
            nc.gpsimd.collective_compute(
                "AllReduce", ALU.add,
                replica_groups=[list(range(NCORE))],
                ins=[sloc[2:4].opt()], outs=[sglob[2:4].opt()],
            )
            rec_raw = sb.tile([NQ, B, H], F32, tag="rec_raw")
            nc.sync.dma_start(
                rec_raw[:, 0:2, :], sglob[0:2].transpose([1, 0, 2]))
            nc.sync.dma_start(
                rec_raw[:, 2:4, :], sglob[2:4].transpose([1, 0, 2]))

            # ---- phase 2: V projection + context, overlapping the collective
            # reverse order so vp[b] can reuse kvnT[b+1]'s memory
            vp = {}
            vp[0] = sb.tile([128, NKT, H, HD], BF16, tag="vpx", name="vp_0")
            for b in (1, 2, 3):
                vp[b] = sb.tile([128, NKT, H, HD], BF16, tag=f"kvnT{b - 1}",
                                name=f"vp_{b}")

            # unnormalized transposed context, bf16 (normalized later)
            ctxTu = sb.tile([128, NDC, B, NQ], BF16, tag="qnT", name="ctxTu")

            def vproj_block(b):
                for r in range(NKT):
                    for nh in range(2):
                        acc = ps.tile([128, 512], F32, tag="mm", bufs=2)
                        for k in range(NDC):
                            nc.tensor.matmul(
                                acc[:],
                                lhsT=kvnT[b][:, k, r, :],
                                rhs=wv_sb[:, k, nh * 512:(nh + 1) * 512],
                                start=(k == 0), stop=(k == NDC - 1),
                            )
                        nc.vector.tensor_copy(
                            out=vp[b][:, r, nh * 8:(nh + 1) * 8, :],
                            in_=acc[:].rearrange("p (g d) -> p g d", g=8),
                        )

            def ctx_block(b):
                for j in range(NDC):
                    ctx_ps = ps.tile([128, NQ], F32, tag="sc", bufs=2)
                    for hh in range(2):
                        h = 2 * j + hh
                        for c in range(NKT):
                            nc.tensor.matmul(
                                ctx_ps[hh * HD:(hh + 1) * HD, :],
                                lhsT=vp[b][:, c, h, :],
                                rhs=exp_all[b][:, c, h, :],
                                start=(c == 0), stop=(c == NKT - 1),
                            )
                    nc.scalar.activation(
                        ctxTu[:, j, b, :], ctx_ps[:], AF.Copy,
                    )

            ctxT = sb.tile([128, NDC, B, NQ], BF16, tag="kpT", name="ctxT")
            out_sb = sb.tile([128, NDC, B, NQ], BF16, tag="wq", bufs=1,
                             name="out_sb")

            def rec_half(half):
                """reciprocal + transpose into rec_bf for batches 2h, 2h+1."""
                b0, b1 = 2 * half, 2 * half + 1
                nc.vector.reciprocal(
                    rec_raw[:, b0:b1 + 1, :], rec_raw[:, b0:b1 + 1, :])
                rec_cast = sb.tile([NQ, 2, H], BF16, tag="rec_cast", bufs=2)
                nc.vector.tensor_copy(
                    out=rec_cast[:], in_=rec_raw[:, b0:b1 + 1, :])
                recT_ps = ps.tile([128, NQ], BF16, tag="tr", bufs=2)
                for i, b in enumerate((b0, b1)):
                    nc.tensor.transpose(
                        recT_ps[32 * (b % 2):32 * (b % 2) + 16, :],
                        rec_cast[:, i, :], ident[:NQ, :NQ],
                    )
                for b in (b0, b1):
                    p0 = 32 * (b % 2)
                    nc.vector.tensor_copy(
                        out=rec_bf[p0:p0 + 16, half, :],
                        in_=recT_ps[p0:p0 + 16, :],
                    )

            def norm_half(half):
                for b in (2 * half, 2 * half + 1):
                    for j in range(NDC):
                        rpp = ps.tile([128, NQ], F32,
                                      tag=("den" if j % 2 else "sc"), bufs=2)
                        nc.tensor.matmul(
                            rpp[:],
                            lhsT=sel_sb[:, b % 2, j, :],
                            rhs=rec_bf[:, b // 2, :],
                            start=True, stop=True,
                        )
                        nc.vector.tensor_mul(
                            ctxT[:, j, b, :], ctxTu[:, j, b, :], rpp[:]
                        )

            def outproj():
                for m in range(NDC):
                    acc = ps.tile([128, 512], F32, tag="mm", bufs=2)
                    for k in range(NDC):
                        nc.tensor.matmul(
                            acc[:, 0:B * NQ],
                            lhsT=wo_sb[:, k, m * 128:(m + 1) * 128],
                            rhs=ctxT[:, k, :, :].rearrange("p b q -> p (b q)"),
                            start=(k == 0), stop=(k == NDC - 1),
                        )
                    nc.vector.tensor_copy(
                        out=out_sb[:, m, :, :],
                        in_=acc[:, 0:B * NQ].rearrange("p (b q) -> p b q", b=B),
                    )
                    nc.sync.dma_start(out_d[:, m], out_sb[:, m])

            # phase 2 with the first-half normalize folded into the middle
            vproj_block(0); ctx_block(0)
            vproj_block(1); ctx_block(1)
            rec_half(0)
            norm_half(0)
            vproj_block(2); ctx_block(2)
            vproj_block(3); ctx_block(3)
            rec_half(1)
            norm_half(1)
            outproj()

    nc.compile()
    return nc


def _prep_in_maps(q, kv, mask, in_proj_w, in_proj_b, out_w, out_b,
                  g_q, b_q, g_kv, b_kv):
    """Host-side prep: fold LN affine + V-bias, shard kv/mask per core.

    Returns (in_maps, bias_total)."""
    q = np.asarray(q, np.float32)
    kv = np.asarray(kv, np.float32)
    mask = np.asarray(mask)
    in_proj_w = np.asarray(in_proj_w, np.float32)
    in_proj_b = np.asarray(in_proj_b, np.float32)
    out_w = np.asarray(out_w, np.float32)
    out_b = np.asarray(out_b, np.float32)
    g_q = np.asarray(g_q, np.float32)
    b_q = np.asarray(b_q, np.float32)
    g_kv = np.asarray(g_kv, np.float32)
    b_kv = np.asarray(b_kv, np.float32)

    Wq, Wk, Wv = in_proj_w[:D], in_proj_w[D:2 * D], in_proj_w[2 * D:]
    bq, bk, bv = in_proj_b[:D], in_proj_b[D:2 * D], in_proj_b[2 * D:]

    # Fold LayerNorm affine into projections: LN(x)*g+b @ W^T + c
    #   = LN(x) @ (W*g)^T + (W@b + c)
    WqT = (Wq * g_q[None, :]).T.astype(ml_dtypes.bfloat16)
    WkT = (Wk * g_kv[None, :]).T.astype(ml_dtypes.bfloat16)
    WvT = (Wv * g_kv[None, :]).T.astype(ml_dtypes.bfloat16)
    bq_eff = (bq + Wq @ b_q).astype(np.float32)
    bk_eff = (bk + Wk @ b_kv).astype(np.float32)
    bv_eff = (bv + Wv @ b_kv).astype(np.float32)
    # V bias passes through softmax unchanged (weights sum to 1): fold into
    # the final output bias on the host.
    WoT = out_w.T.astype(ml_dtypes.bfloat16)
    bias_total = (out_b + out_w @ bv_eff).astype(np.float32)

    # per-query key mask; all-zero mask rows attend everywhere
    kv16 = kv.astype(ml_dtypes.bfloat16)
    allowed = (mask != 0)
    has_any = allowed.any(axis=-1, keepdims=True)
    eff = np.where(has_any, allowed, True)  # [B, NQ, HW] bool

    # selector for the reciprocal partition-broadcast:
    # sel[p, a, j, c] = 1 iff p == 32a + 2j + c//64  (a = b%2)
    sel = np.zeros((128, 2, NDC, 128), dtype=ml_dtypes.bfloat16)
    for a in range(2):
        for j in range(NDC):
            for cb in range(2):
                sel[32 * a + 2 * j + cb, a, j, cb * 64:(cb + 1) * 64] = 1.0

    common = {
        "q": np.ascontiguousarray(q.astype(ml_dtypes.bfloat16)),
        "wqT": np.ascontiguousarray(WqT.reshape(NDC, 128, D).transpose(1, 0, 2)),
        "wkT": np.ascontiguousarray(WkT.reshape(NDC, 128, D).transpose(1, 0, 2)),
        "wvT": np.ascontiguousarray(WvT.reshape(NDC, 128, D).transpose(1, 0, 2)),
        "woT": np.ascontiguousarray(WoT.reshape(NDC, 128, D).transpose(1, 0, 2)),
        "biasq": np.ascontiguousarray(bq_eff.reshape(NDC, 128).T),
        "biask": np.ascontiguousarray(bk_eff.reshape(NDC, 128).T),
        "sel": sel,
        "ident": np.eye(128, dtype=ml_dtypes.bfloat16),
    }
    in_maps = []
    for c in range(NCORE):
        sl = slice(c * KC, (c + 1) * KC)
        kv_c = kv16[:, sl, :].reshape(B, NKT, 128, D)
        # mask slice -> [B, 128, NKT, NQ] bf16 (keysub-tile on partitions)
        m_c = eff[:, :, sl].transpose(0, 2, 1).reshape(B, NKT, 128, NQ)
        m_c = m_c.transpose(0, 2, 1, 3).astype(ml_dtypes.bfloat16)
        in_maps.append({
            **common,
            "kv": np.ascontiguousarray(kv_c),
            "maskT": np.ascontiguousarray(m_c),
        })
    return in_maps, bias_total


def kernel(q, kv, mask, in_proj_w, in_proj_b, out_w, out_b, g_q, b_q, g_kv, b_kv):
    in_maps, bias_total = _prep_in_maps(
        q, kv, mask, in_proj_w, in_proj_b, out_w, out_b, g_q, b_q, g_kv, b_kv
    )
    if "nc" not in _compiled:
        _compiled["nc"] = _build()
    nc = _compiled["nc"]

    res = run_bass_kernel_spmd(nc, in_maps, core_ids=list(range(NCORE)))

    out = np.zeros((B, NQ, D), np.float32)
    for c in range(NCORE):
        part = res.results[c]["out"].astype(np.float32)
        out += part.transpose(2, 3, 1, 0).reshape(B, NQ, D)
    out += bias_total[None, None, :]
    return out
